# revision 29
# baseline (speedup 1.0000x reference)
"""Trainium2 Bass kernel for nn_ACS (4-branch conv block + top-k channel select).

Strategy:
- Host: top-k of c_score; gather ONLY the 512 surviving output channels;
  fold fuse_weight, all BN affines and the sigmoid scale into conv
  weights / per-channel eviction affines. Output channel permutation is
  applied host-side after gather (free).
- Device (SPMD, 8 cores, 2 images each; no collectives):
  * activations live in a zero-padded [C, 58*58]-style layout; 3x3 convs
    run as 9 shifted matmuls accumulating in PSUM (fp32r: full-rate
    fp32-rounded matmul dtype).
  * branch0 = 3x3 conv; branch1 = 1x1 conv; branch2 = 1x1 conv ->
    BN-affine eviction -> 3x3 conv (+bias evict); branch3 = 1x1 conv ->
    BN-affine eviction -> separable 3x3 sum-pool on VectorE -> affine evict.
  * branch0's ragged tail tile is topped up with branch1 channels
    (center-tap only) so the 128-wide PE columns stay productive.
  * per-image work is split in two 28-row slabs, chunked into 7/8-row
    matmul groups (free dim 406/464 <= one PSUM bank).
"""

import numpy as np

import concourse.mybir as mybir
import concourse.tile as tile
from concourse import bacc
from concourse.bass_utils import run_bass_kernel_spmd

# ---- problem constants (hardcoded per spec) ----
B, C, H, W = 16, 512, 56, 56
MID = 256
NCORES = 8
BL = B // NCORES  # images per core
WP = H + 2  # padded row width 58
SLAB_ROWS = 28
BUF = 30 * WP + 2  # slab buffer free size (30 padded rows + 2 slack) = 1742
EPS = 1e-5
CENTER_FIRST = [4, 0, 1, 2, 3, 5, 6, 7, 8]  # tap order: center tap starts PSUM

F32 = mybir.dt.float32
F32R = mybir.dt.float32r
AF = mybir.ActivationFunctionType

PROFILE = False
LAST_EXEC_NS = None
_CACHE = {}


def _tiles_of(M):
    out = []
    o = 0
    while o < M:
        out.append((o, min(128, M - o)))
        o += 128
    return out


def _e_split(M0):
    """b0 tail: use tap-expansion when the ragged remainder is small."""
    rem9 = M0 % 128
    use_e = 0 < rem9 <= 28
    return (M0 - rem9, rem9) if use_e else (M0, 0)


def _par_cols(counts):
    M0, M1, M2, M3 = counts
    n2, n3 = len(_tiles_of(M2)), len(_tiles_of(M3))
    cols = {"s31": 0, "t31": 2}
    c = 4
    cols["a_s1"] = c
    c += n3
    cols["a_t1"] = c
    c += n3
    cols["bias2"] = c
    c += n2
    cols["pscale"] = c
    c += n3
    cols["pbias"] = c
    c += n3
    return cols, c


# ---------------- host-side folding ----------------

def _bn_fold(p):
    g, b, m, v = [p[i].astype(np.float64) for i in range(4)]
    s = g / np.sqrt(v + EPS)
    t = b - m * s
    return s.astype(np.float32), t.astype(np.float32)


def _prep(w_main, w_1x1, w31, bn31, w33, bn33, wa1, bna1, bna2, fuse_weight, c_score):
    f = [float(fuse_weight[i]) for i in range(4)]
    s31, t31 = _bn_fold(bn31)
    s33, t33 = _bn_fold(bn33)
    sa1, ta1 = _bn_fold(bna1)
    sa2, ta2 = _bn_fold(bna2)

    ind = np.argsort(-c_score, kind="stable")[:C]
    scale = (1.0 / (1.0 + np.exp(-c_score.astype(np.float64))))[ind].astype(np.float32)

    js = {b_: [] for b_ in range(4)}
    cs = {b_: [] for b_ in range(4)}
    for j, gi in enumerate(ind):
        b_ = int(gi) // 256
        js[b_].append(j)
        cs[b_].append(int(gi) % 256)
    c0, c1, c2, c3 = [np.array(cs[i], np.int64) for i in range(4)]
    sc = [scale[np.array(js[i], np.int64)] if js[i] else np.zeros(0, np.float32) for i in range(4)]

    W0 = w_main[c0] * f[0] * sc[0][:, None, None, None]            # [M0,C,3,3]
    W1 = w_1x1[c1, :, 0, 0] * f[1] * sc[1][:, None]                # [M1,C]
    W31 = w31[:, :, 0, 0] * f[2]                                   # [MID,C]
    W33 = w33[c2] * (s33[c2] * sc[2])[:, None, None, None]         # [M2,MID,3,3]
    bias2 = t33[c2] * sc[2]
    Wa1 = wa1[c3, :, 0, 0] * f[3]                                  # [M3,C]
    a_s1, a_t1 = sa1[c3], ta1[c3]
    pscale = sa2[c3] * sc[3] / 9.0
    pbias = ta2[c3] * sc[3]

    jperm = np.array(js[0] + js[1] + js[2] + js[3], dtype=np.int64)
    counts = (len(c0), len(c1), len(c2), len(c3))
    M0, M1, M2, M3 = counts

    # branch0: full 128-wide tiles; small ragged tail handled via tap
    # expansion (e-channels) instead of a nearly-empty 9-tap tile
    rem9 = M0 % 128
    use_e = 0 < rem9 <= 28
    M0F = M0 - rem9 if use_e else M0
    W0F = W0[:M0F]
    if use_e:
        W0T = W0[M0F:]  # [rem9, C, 3, 3]
        # e-channel 1x1 weights: channel (t*rem9 + j) = tap t of tail ch j
        Ew = np.ascontiguousarray(
            W0T.transpose(2, 3, 0, 1).reshape(9 * rem9, C)
        )
        # selector for the tail conv: lhsT[i, t, j] = 1 iff i == t*rem9+j
        sel = np.zeros((128, 9, rem9), np.float32)
        for t_ in range(9):
            for j_ in range(rem9):
                sel[t_ * rem9 + j_, t_, j_] = 1.0

    def pack_kpm(Wmat, ktiles):
        M = Wmat.shape[0]
        return np.ascontiguousarray(
            Wmat.T.reshape(ktiles, 128, M).transpose(1, 0, 2).reshape(128, ktiles * M)
        ).astype(np.float32)

    def pack_ktpm(Wconv, ktiles):
        M = Wconv.shape[0]
        return np.ascontiguousarray(
            Wconv.transpose(1, 2, 3, 0)
            .reshape(ktiles, 128, 9, M)
            .transpose(1, 0, 2, 3)
            .reshape(128, ktiles * 9 * M)
        ).astype(np.float32)

    # can the e-channels ride in branch3's ragged tile matmuls?
    NE = 9 * rem9 if use_e else 0
    rem3 = M3 % 128
    n3full = M3 - rem3
    EA = 32 * ((NE + 31) // 32)
    combine = use_e and rem3 > 0 and EA + rem3 <= 128

    arrs = {
        "W31d": np.ascontiguousarray(
            W31.T.reshape(4, 128, 2, 128).transpose(1, 2, 0, 3).reshape(128, 2 * 4 * 128)
        ).astype(np.float32)
    }
    if M0F:
        arrs["W0d"] = pack_ktpm(W0F, 4)
    if use_e:
        arrs["seld"] = np.ascontiguousarray(sel.reshape(128, 9 * rem9))
        if not combine:
            arrs["Ewd"] = pack_kpm(Ew, 4)
    if combine:
        Wride = np.zeros((EA + rem3, C), np.float32)
        Wride[:NE] = Ew
        Wride[EA:] = Wa1[n3full:]
        arrs["Wrided"] = pack_kpm(Wride, 4)
    if M1:
        arrs["W1d"] = pack_kpm(W1, 4)
    if M2:
        arrs["W33d"] = pack_ktpm(W33, 2)
    if (n3full if combine else M3):
        arrs["Wa1d"] = pack_kpm(Wa1[: n3full if combine else M3], 4)

    cols, ncol = _par_cols(counts)
    par = np.zeros((128, ncol), np.float32)
    par[:, cols["s31"]], par[:, cols["s31"] + 1] = s31[:128], s31[128:]
    par[:, cols["t31"]], par[:, cols["t31"] + 1] = t31[:128], t31[128:]
    for o, (mo, mw) in enumerate(_tiles_of(M3)):
        par[:mw, cols["a_s1"] + o] = a_s1[mo : mo + mw]
        par[:mw, cols["a_t1"] + o] = a_t1[mo : mo + mw]
        par[:mw, cols["pscale"] + o] = pscale[mo : mo + mw]
        par[:mw, cols["pbias"] + o] = pbias[mo : mo + mw]
    for o, (mo, mw) in enumerate(_tiles_of(M2)):
        par[:mw, cols["bias2"] + o] = bias2[mo : mo + mw]
    arrs["par"] = par
    return arrs, counts, jperm


# ---------------- device graph ----------------

def _build(counts):
    M0, M1, M2, M3 = counts
    cols, ncol = _par_cols(counts)
    M0F, rem9 = _e_split(M0)
    NE = 9 * rem9  # e-channel count
    rem3 = M3 % 128
    n3full = M3 - rem3
    EA = 32 * ((NE + 31) // 32)
    combine = rem9 > 0 and rem3 > 0 and EA + rem3 <= 128
    M3W = n3full if combine else M3  # channels served by the plain wa1 tiles
    n3tiles = _tiles_of(M3)
    nc = bacc.Bacc("TRN2", target_bir_lowering=False, debug=False, num_devices=NCORES)

    x_ext = nc.dram_tensor("x", [BL, C, H, W], F32R, kind="ExternalInput")
    W0_ext = nc.dram_tensor("W0d", [128, 4 * 9 * M0F], F32R, kind="ExternalInput") if M0F else None
    Ew_ext = nc.dram_tensor("Ewd", [128, 4 * NE], F32R, kind="ExternalInput") if rem9 and not combine else None
    sel_ext = nc.dram_tensor("seld", [128, NE], F32R, kind="ExternalInput") if rem9 else None
    Wr_ext = nc.dram_tensor("Wrided", [128, 4 * (EA + rem3)], F32R, kind="ExternalInput") if combine else None
    W1_ext = nc.dram_tensor("W1d", [128, 4 * M1], F32R, kind="ExternalInput") if M1 else None
    W31_ext = nc.dram_tensor("W31d", [128, 4 * MID], F32R, kind="ExternalInput")
    W33_ext = nc.dram_tensor("W33d", [128, 2 * 9 * M2], F32R, kind="ExternalInput") if M2 else None
    Wa1_ext = nc.dram_tensor("Wa1d", [128, 4 * M3W], F32R, kind="ExternalInput") if M3W else None
    par_ext = nc.dram_tensor("par", [128, ncol], F32, kind="ExternalInput")
    out_ext = nc.dram_tensor("out", [BL, C, H, W], F32, kind="ExternalOutput")

    off1, off2, off3 = M0, M0 + M1, M0 + M1 + M2

    with tile.TileContext(nc) as tc:
        with (
            tc.tile_pool(name="wpool", bufs=1) as wpool,
            tc.tile_pool(name="acts", bufs=1) as acts,
            tc.tile_pool(name="stage", bufs=3) as stage,
            tc.tile_pool(name="xstage", bufs=16) as xstage,
            tc.tile_pool(name="slabstage", bufs=1) as slabstage,
            tc.tile_pool(name="ps", bufs=8, space="PSUM") as ps,
        ):
            # ---- persistent weights; only wt31 loads before slab-0's x ----
            part = wpool.tile([128, ncol], F32)
            wt31 = wpool.tile([128, 2, 4, 128], F32R)
            wta1 = wpool.tile([128, 4, M3W], F32R, name="wta1") if M3W else None
            wtr = wpool.tile([128, 4, EA + rem3], F32R, name="wtr") if combine else None
            wt0 = wpool.tile([128, 4, 9, M0F], F32R, name="wt0") if M0F else None
            wte = wpool.tile([128, 4, NE], F32R, name="wte") if (rem9 and not combine) else None
            selt = wpool.tile([128, 9, rem9], F32R, name="selt") if rem9 else None
            wt1 = wpool.tile([128, 4, M1], F32R, name="wt1") if M1 else None
            wt33 = wpool.tile([128, 2, 9, M2], F32R, name="wt33") if M2 else None
            w31ap = W31_ext.ap().rearrange("p (o k m) -> p o k m", o=2, k=4)
            nc.sync.dma_start(wt31[:, 0], w31ap[:, 0])
            nc.sync.dma_start(wt31[:, 1], w31ap[:, 1])
            nc.gpsimd.dma_start(part[:], par_ext.ap())

            # small weights ride the gpsimd SWDGE lane, available early
            if combine:
                nc.gpsimd.dma_start(wtr[:], Wr_ext.ap().rearrange("p (k m) -> p k m", k=4))
            if M3W:
                nc.gpsimd.dma_start(wta1[:], Wa1_ext.ap().rearrange("p (k m) -> p k m", k=4))
            if rem9 and not combine:
                nc.gpsimd.dma_start(wte[:], Ew_ext.ap().rearrange("p (k m) -> p k m", k=4))
            if rem9:
                nc.gpsimd.dma_start(selt[:], sel_ext.ap().rearrange("p (t m) -> p t m", t=9))
            if M1:
                nc.gpsimd.dma_start(wt1[:], W1_ext.ap().rearrange("p (k m) -> p k m", k=4))

            def load_big_weights():
                # emitted after slab-0's x pieces so the scalar ring serves
                # the first chunks before streaming the large conv weights
                if M0F:
                    nc.scalar.dma_start(wt0[:], W0_ext.ap().rearrange("p (k t m) -> p k t m", k=4, t=9))
                if M2:
                    nc.scalar.dma_start(wt33[:], W33_ext.ap().rearrange("p (k t m) -> p k t m", k=2, t=9))

            # ---- persistent activation buffers (2 slab slots each) ----
            xt = [[acts.tile([128, BUF], F32R, name=f"xt{k}{s}", tag=f"x{k}s{s}") for s in range(2)] for k in range(4)]
            z1 = [[acts.tile([128, BUF], F32R, name=f"z1{k}{s}", tag=f"z{k}s{s}") for s in range(2)] for k in range(2)]
            za = [[acts.tile([128, BUF], F32, name=f"za{o}{s}", tag=f"za{o}s{s}") for s in range(2)] for o in range(len(n3tiles))]
            ze = [acts.tile([128, BUF], F32R, name=f"ze{s}", tag=f"zes{s}") for s in range(2)] if rem9 else None
            hs = [acts.tile([128, BUF], F32, name="hs0", tag="hs0")] * max(1, len(n3tiles))

            # zero only the pad regions (row pads, col pads, slack), not the
            # whole buffers: three tiny memsets per buffer, split over engines.
            def pad_memsets(t, eng):
                a = t[:].bitcast(mybir.dt.uint32)
                eng.memset(a[:, 0:59], 0)  # slack + row 0
                # col pads: w in {0,57} of every row == flat {58r, 58r+1}
                eng.memset(a[:, 0 : 30 * WP].rearrange("p (r w) -> p r w", w=WP)[:, :, 0:2], 0)
                eng.memset(a[:, 29 * WP + 1 : BUF], 0)  # row 29 + tail slack

            def emit_pad_memsets(sidx):
                for group in (xt, z1, za, [ze] if rem9 else []):
                    for pair in group:
                        pad_memsets(pair[sidx], nc.vector)

            emit_pad_memsets(0)  # slot-1 pads are zeroed during slab 0 (see loop)

            OUT_STARTS = [1, 8, 15, 22]  # slab-local output row starts (7 rows)

            def evict_to_out(acc, mw, segs, b, g0):
                """PSUM rows [0,mw) -> valid cols -> stage; then one DMA per
                (p_lo, p_hi, ch0) segment (PSUM reads must start at part 0)."""
                st = stage.tile([mw, 7 * 56], F32, name="st", tag="st")
                src = acc[0:mw].rearrange("p (r w) -> p r w", w=WP)[:, :, 1:57]
                dst = st[:].rearrange("p (r w) -> p r w", w=56)
                nc.scalar.activation(dst, src, AF.Copy)
                oeng = nc.sync if (g0 // 7) % 2 == 0 else nc.scalar
                for (p_lo, p_hi, ch0) in segs:
                    oeng.dma_start(
                        out_ext.ap()[b, ch0 : ch0 + p_hi - p_lo, g0 : g0 + 7, :],
                        st[p_lo:p_hi].rearrange("p (r w) -> p r w", w=56),
                    )

            def emit_x(b, s):
                """DMA one x slab into compact staging (both HWDGE rings) and
                place into the padded layout on DVE/ACT."""
                x_pieces = [(1, 7), (8, 7), (15, 7), (22, 8)] if s == 0 else [(0, 7), (7, 7), (14, 7), (21, 8)]
                xs_tiles = {}
                for ci, (lp, pn) in enumerate(x_pieces):
                    for k in range(4):
                        xs = xstage.tile([128, 8 * 56], F32R, name="xs", tag="xs")
                        xs_tiles[(ci, k)] = xs
                        deng = nc.sync if k < 2 else nc.scalar
                        deng.dma_start(
                            xs[:, 0 : pn * 56],
                            x_ext.ap()[b, 128 * k : 128 * (k + 1),
                                       SLAB_ROWS * s + lp - 1 : SLAB_ROWS * s + lp - 1 + pn, :],
                        )
                return x_pieces, xs_tiles

            def emit_copies(b, s, x_pieces, xs_tiles):
                for ci, (lp, pn) in enumerate(x_pieces):
                    for k in range(4):
                        xs = xs_tiles[(ci, k)]
                        dst = xt[k][s][:, lp * WP + 2 : lp * WP + 2 + pn * WP].rearrange(
                            "p (r w) -> p r w", w=WP
                        )[:, :, 0:56]
                        srcv = xs[:, 0 : pn * 56].rearrange("p (r w) -> p r w", w=56)
                        if k % 2 == 0:
                            nc.vector.tensor_copy(dst, srcv)
                        else:
                            nc.scalar.activation(dst, srcv, AF.Copy)

            def emit_mids(b, s, xs_tiles):
                mid_chunks = [(1, 7), (8, 7), (15, 7), (22, 8)] if s == 0 else [(0, 7), (7, 7), (14, 7), (21, 8)]
                for ci, (l0c, nr) in enumerate(mid_chunks):
                    NV = nr * 56

                    def mid_conv(weight_ap, mw_, dstbuf, scale_c=None, bias_c=None):
                        acc = ps.tile([mw_, NV], F32, name="acc", tag="ps")
                        for k in range(4):
                            nc.tensor.matmul(
                                acc[:], weight_ap(k), xs_tiles[(ci, k)][:, 0:NV],
                                start=(k == 0), stop=(k == 3),
                            )
                        dst = dstbuf[:mw_, l0c * WP + 2 : l0c * WP + 2 + nr * WP].rearrange(
                            "p (r w) -> p r w", w=WP
                        )[:, :, 0:56]
                        srcv = acc[:].rearrange("p (r w) -> p r w", w=56)
                        if scale_c is None:
                            nc.scalar.activation(dst, srcv, AF.Copy)
                        else:
                            nc.scalar.activation(
                                dst, srcv, AF.Identity,
                                bias=part[:mw_, bias_c : bias_c + 1],
                                scale=part[:mw_, scale_c : scale_c + 1],
                            )

                    for o in range(2):
                        mid_conv(lambda k, o=o: wt31[:, o, k, :], 128, z1[o][s],
                                 cols["s31"] + o, cols["t31"] + o)
                    for o, (mo, mw) in enumerate(_tiles_of(M3W)):
                        mid_conv(lambda k, mo=mo, mw=mw: wta1[:, k, mo : mo + mw], mw,
                                 za[o][s], cols["a_s1"] + o, cols["a_t1"] + o)
                    if combine:
                        # one group computes the e-channels (rows 0:NE) and
                        # branch3's ragged tile (rows EA:EA+rem3, 32-aligned)
                        o = len(n3tiles) - 1
                        acc = ps.tile([EA + rem3, NV], F32, name="acc", tag="ps")
                        for k in range(4):
                            nc.tensor.matmul(
                                acc[:], wtr[:, k, :], xs_tiles[(ci, k)][:, 0:NV],
                                start=(k == 0), stop=(k == 3),
                            )
                        dste = ze[s][:NE, l0c * WP + 2 : l0c * WP + 2 + nr * WP].rearrange(
                            "p (r w) -> p r w", w=WP
                        )[:, :, 0:56]
                        nc.scalar.activation(
                            dste, acc[0:NE].rearrange("p (r w) -> p r w", w=56), AF.Copy
                        )
                        dsta = za[o][s][:rem3, l0c * WP + 2 : l0c * WP + 2 + nr * WP].rearrange(
                            "p (r w) -> p r w", w=WP
                        )[:, :, 0:56]
                        nc.scalar.activation(
                            dsta, acc[EA : EA + rem3].rearrange("p (r w) -> p r w", w=56),
                            AF.Identity,
                            bias=part[:rem3, cols["a_t1"] + o : cols["a_t1"] + o + 1],
                            scale=part[:rem3, cols["a_s1"] + o : cols["a_s1"] + o + 1],
                        )
                    elif rem9:
                        mid_conv(lambda k: wte[:, k, :], NE, ze[s])

            def emit_pool(b, s):
                add = mybir.AluOpType.add
                for o, (mo, mw) in enumerate(n3tiles):
                    zz = za[o][s]
                    hh = hs[o]
                    nc.vector.tensor_tensor(
                        hh[:mw, 1 : BUF - 1], zz[:mw, 0 : BUF - 2], zz[:mw, 2:BUF], add
                    )
                    nc.vector.tensor_tensor(
                        hh[:mw, 1 : BUF - 1], hh[:mw, 1 : BUF - 1], zz[:mw, 1 : BUF - 1], add
                    )

                    def v3(buf, base, mwl=mw):
                        return buf[:mwl, base : base + 28 * WP].rearrange(
                            "p (r w) -> p r w", w=WP
                        )[:, :, 0:56]

                    nc.vector.tensor_tensor(v3(zz, 60), v3(hh, 2), v3(hh, 118), add)
                    nc.vector.tensor_tensor(v3(zz, 60), v3(zz, 60), v3(hh, 60), add)
                    st = slabstage.tile([mw, 28 * 56], F32, name="sst", tag="sst")
                    nc.scalar.activation(
                        st[:].rearrange("p (r w) -> p r w", w=56),
                        v3(zz, 60), AF.Identity,
                        bias=part[:mw, cols["pbias"] + o : cols["pbias"] + o + 1],
                        scale=part[:mw, cols["pscale"] + o : cols["pscale"] + o + 1],
                    )
                    g0 = SLAB_ROWS * s
                    nc.sync.dma_start(
                        out_ext.ap()[b, off3 + mo : off3 + mo + mw, g0 : g0 + 28, :],
                        st[:].rearrange("p (r w) -> p r w", w=56),
                    )

            def emit_b0F(b, s, l0c):
                q0 = l0c * WP + 1
                N = 7 * WP
                g0 = l0c - 1 + SLAB_ROWS * s
                for (mo, mw) in _tiles_of(M0F):
                    acc = ps.tile([mw, N], F32, name="acc", tag="ps")
                    first = True
                    for t in CENTER_FIRST:
                        dh, dw = t // 3, t % 3
                        qr = q0 + (dh - 1) * WP + (dw - 1)
                        for k in range(4):
                            nc.tensor.matmul(
                                acc[:], wt0[:, k, t, mo : mo + mw],
                                xt[k][s][:, qr : qr + N],
                                start=first, stop=(t == 8 and k == 3),
                            )
                            first = False
                    evict_to_out(acc, mw, [(0, mw, mo)], b, g0)

            def emit_tail(b, s, l0c):
                if not rem9:
                    return
                q0 = l0c * WP + 1
                N = 7 * WP
                g0 = l0c - 1 + SLAB_ROWS * s
                acc = ps.tile([rem9, N], F32, name="acc", tag="ps")
                first = True
                for t in CENTER_FIRST:
                    dh, dw = t // 3, t % 3
                    qr = q0 + (dh - 1) * WP + (dw - 1)
                    nc.tensor.matmul(
                        acc[:], selt[:NE, t, :], ze[s][:NE, qr : qr + N],
                        start=first, stop=(t == 8),
                    )
                    first = False
                evict_to_out(acc, rem9, [(0, rem9, M0F)], b, g0)

            def emit_b1(b, s, l0c):
                q0 = l0c * WP + 1
                N = 7 * WP
                g0 = l0c - 1 + SLAB_ROWS * s
                for (mo, mw) in _tiles_of(M1):
                    acc = ps.tile([mw, N], F32, name="acc", tag="ps")
                    for k in range(4):
                        nc.tensor.matmul(
                            acc[:], wt1[:, k, mo : mo + mw], xt[k][s][:, q0 : q0 + N],
                            start=(k == 0), stop=(k == 3),
                        )
                    evict_to_out(acc, mw, [(0, mw, off1 + mo)], b, g0)

            def emit_b2c2(b, s, l0c):
                q0 = l0c * WP + 1
                N = 7 * WP
                g0 = l0c - 1 + SLAB_ROWS * s
                for o, (mo, mw) in enumerate(_tiles_of(M2)):
                    acc = ps.tile([mw, N], F32, name="acc", tag="ps")
                    first = True
                    for t in CENTER_FIRST:
                        dh, dw = t // 3, t % 3
                        qr = q0 + (dh - 1) * WP + (dw - 1)
                        for k in range(2):
                            nc.tensor.matmul(
                                acc[:], wt33[:, k, t, mo : mo + mw],
                                z1[k][s][:, qr : qr + N],
                                start=first, stop=(t == 8 and k == 1),
                            )
                            first = False
                    st = stage.tile([mw, 7 * 56], F32, name="st", tag="st")
                    nc.scalar.activation(
                        st[:].rearrange("p (r w) -> p r w", w=56),
                        acc[:].rearrange("p (r w) -> p r w", w=WP)[:, :, 1:57],
                        AF.Identity,
                        bias=part[:mw, cols["bias2"] + o : cols["bias2"] + o + 1],
                    )
                    oeng = nc.sync if (g0 // 7) % 2 == 1 else nc.scalar
                    oeng.dma_start(
                        out_ext.ap()[b, off2 + mo : off2 + mo + mw, g0 : g0 + 7, :],
                        st[:].rearrange("p (r w) -> p r w", w=56),
                    )

            # ---- pipelined emission schedule ----
            slabs = [(b, s) for b in range(BL) for s in range(2)]
            pieces, tiles = emit_x(*slabs[0])
            xq = {0: (pieces, tiles)}
            for idx, (b, s) in enumerate(slabs):
                x_pieces, xs_tiles = xq.pop(idx)
                if idx == 0:
                    load_big_weights()
                if idx == 1:
                    emit_pad_memsets(1)
                emit_copies(b, s, x_pieces, xs_tiles)
                emit_mids(b, s, xs_tiles)
                emit_pool(b, s)
                if idx + 1 < len(slabs):
                    xq[idx + 1] = emit_x(*slabs[idx + 1])
                if idx == 0:
                    # slab 0: run small-weight branches first so the large
                    # conv weights have time to stream in behind the x pieces
                    for l0c in OUT_STARTS:
                        emit_b1(b, s, l0c)
                    for l0c in OUT_STARTS:
                        emit_tail(b, s, l0c)
                    for l0c in OUT_STARTS:
                        emit_b2c2(b, s, l0c)
                    for l0c in OUT_STARTS:
                        emit_b0F(b, s, l0c)
                else:
                    for l0c in OUT_STARTS:
                        emit_b0F(b, s, l0c)
                        emit_tail(b, s, l0c)
                        emit_b1(b, s, l0c)
                    for l0c in OUT_STARTS:
                        emit_b2c2(b, s, l0c)

    nc.compile()
    return nc


# ---------------- entry point ----------------

def kernel(x, w_main, w_1x1, w31, bn31, w33, bn33, wa1, bna1, bna2, fuse_weight, c_score):
    global LAST_EXEC_NS
    arrs, counts, jperm = _prep(
        w_main, w_1x1, w31, bn31, w33, bn33, wa1, bna1, bna2, fuse_weight, c_score
    )
    if counts not in _CACHE:
        _CACHE[counts] = _build(counts)
    nc = _CACHE[counts]

    x = np.ascontiguousarray(x, dtype=np.float32)
    in_maps = []
    for i in range(NCORES):
        m = {"x": np.ascontiguousarray(x[BL * i : BL * (i + 1)])}
        m.update(arrs)
        in_maps.append(m)

    res = run_bass_kernel_spmd(nc, in_maps, list(range(NCORES)), trace=PROFILE)
    LAST_EXEC_NS = res.exec_time_ns

    full = np.empty((B, C, H, W), np.float32)
    for i in range(NCORES):
        full[BL * i : BL * (i + 1)] = res.results[i]["out"]
    out = np.empty_like(full)
    out[:, jperm] = full
    return out


# revision 30
# speedup vs baseline: 1.1631x; 1.1631x over previous
"""Trainium2 Bass kernel for nn_ACS (4-branch conv block + top-k channel select).

Strategy:
- Host: top-k of c_score; gather ONLY the 512 surviving output channels;
  fold fuse_weight, all BN affines and the sigmoid scale into conv
  weights / per-channel eviction affines. Output channel permutation is
  applied host-side after gather (free).
- Device (SPMD, 8 cores, 2 images each; no collectives):
  * activations live in a zero-padded [C, 58*58]-style layout; 3x3 convs
    run as 9 shifted matmuls accumulating in PSUM (fp32r: full-rate
    fp32-rounded matmul dtype).
  * branch0 = 3x3 conv; branch1 = 1x1 conv; branch2 = 1x1 conv ->
    BN-affine eviction -> 3x3 conv (+bias evict); branch3 = 1x1 conv ->
    BN-affine eviction -> separable 3x3 sum-pool on VectorE -> affine evict.
  * branch0's ragged tail tile is topped up with branch1 channels
    (center-tap only) so the 128-wide PE columns stay productive.
  * per-image work is split in two 28-row slabs, chunked into 7/8-row
    matmul groups (free dim 406/464 <= one PSUM bank).
"""

import numpy as np

import concourse.mybir as mybir
import concourse.tile as tile
from concourse import bacc
from concourse.bass_utils import run_bass_kernel_spmd

# ---- problem constants (hardcoded per spec) ----
B, C, H, W = 16, 512, 56, 56
MID = 256
NCORES = 8
BL = B // NCORES  # images per core
WP = H + 2  # padded row width 58
SLAB_ROWS = 28
BUF = 30 * WP + 2  # slab buffer free size (30 padded rows + 2 slack) = 1742
EPS = 1e-5
CENTER_FIRST = [4, 0, 1, 2, 3, 5, 6, 7, 8]  # tap order: center tap starts PSUM

F32 = mybir.dt.float32
F32R = mybir.dt.float32r
AF = mybir.ActivationFunctionType

PROFILE = False
LAST_EXEC_NS = None
_CACHE = {}


def _tiles_of(M):
    out = []
    o = 0
    while o < M:
        out.append((o, min(128, M - o)))
        o += 128
    return out


def _e_split(M0):
    """b0 tail: use tap-expansion when the ragged remainder is small."""
    rem9 = M0 % 128
    use_e = 0 < rem9 <= 28
    return (M0 - rem9, rem9) if use_e else (M0, 0)


def _par_cols(counts):
    M0, M1, M2, M3 = counts
    n2, n3 = len(_tiles_of(M2)), len(_tiles_of(M3))
    cols = {"s31": 0, "t31": 2}
    c = 4
    cols["a_s1"] = c
    c += n3
    cols["a_t1"] = c
    c += n3
    cols["bias2"] = c
    c += n2
    cols["pscale"] = c
    c += n3
    cols["pbias"] = c
    c += n3
    return cols, c


# ---------------- host-side folding ----------------

def _bn_fold(p):
    g, b, m, v = [p[i].astype(np.float64) for i in range(4)]
    s = g / np.sqrt(v + EPS)
    t = b - m * s
    return s.astype(np.float32), t.astype(np.float32)


def _prep(w_main, w_1x1, w31, bn31, w33, bn33, wa1, bna1, bna2, fuse_weight, c_score):
    f = [float(fuse_weight[i]) for i in range(4)]
    s31, t31 = _bn_fold(bn31)
    s33, t33 = _bn_fold(bn33)
    sa1, ta1 = _bn_fold(bna1)
    sa2, ta2 = _bn_fold(bna2)

    ind = np.argsort(-c_score, kind="stable")[:C]
    scale = (1.0 / (1.0 + np.exp(-c_score.astype(np.float64))))[ind].astype(np.float32)

    js = {b_: [] for b_ in range(4)}
    cs = {b_: [] for b_ in range(4)}
    for j, gi in enumerate(ind):
        b_ = int(gi) // 256
        js[b_].append(j)
        cs[b_].append(int(gi) % 256)
    c0, c1, c2, c3 = [np.array(cs[i], np.int64) for i in range(4)]
    sc = [scale[np.array(js[i], np.int64)] if js[i] else np.zeros(0, np.float32) for i in range(4)]

    W0 = w_main[c0] * f[0] * sc[0][:, None, None, None]            # [M0,C,3,3]
    W1 = w_1x1[c1, :, 0, 0] * f[1] * sc[1][:, None]                # [M1,C]
    W31 = w31[:, :, 0, 0] * f[2]                                   # [MID,C]
    W33 = w33[c2] * (s33[c2] * sc[2])[:, None, None, None]         # [M2,MID,3,3]
    bias2 = t33[c2] * sc[2]
    Wa1 = wa1[c3, :, 0, 0] * f[3]                                  # [M3,C]
    a_s1, a_t1 = sa1[c3], ta1[c3]
    pscale = sa2[c3] * sc[3] / 9.0
    pbias = ta2[c3] * sc[3]

    jperm = np.array(js[0] + js[1] + js[2] + js[3], dtype=np.int64)
    counts = (len(c0), len(c1), len(c2), len(c3))
    M0, M1, M2, M3 = counts

    # branch0: full 128-wide tiles; small ragged tail handled via tap
    # expansion (e-channels) instead of a nearly-empty 9-tap tile
    rem9 = M0 % 128
    use_e = 0 < rem9 <= 28
    M0F = M0 - rem9 if use_e else M0
    W0F = W0[:M0F]
    if use_e:
        W0T = W0[M0F:]  # [rem9, C, 3, 3]
        # e-channel 1x1 weights: channel (t*rem9 + j) = tap t of tail ch j
        Ew = np.ascontiguousarray(
            W0T.transpose(2, 3, 0, 1).reshape(9 * rem9, C)
        )
        # selector for the tail conv: lhsT[i, t, j] = 1 iff i == t*rem9+j
        sel = np.zeros((128, 9, rem9), np.float32)
        for t_ in range(9):
            for j_ in range(rem9):
                sel[t_ * rem9 + j_, t_, j_] = 1.0

    def pack_kpm(Wmat, ktiles):
        M = Wmat.shape[0]
        return np.ascontiguousarray(
            Wmat.T.reshape(ktiles, 128, M).transpose(1, 0, 2).reshape(128, ktiles * M)
        ).astype(np.float32)

    def pack_ktpm(Wconv, ktiles):
        M = Wconv.shape[0]
        return np.ascontiguousarray(
            Wconv.transpose(1, 2, 3, 0)
            .reshape(ktiles, 128, 9, M)
            .transpose(1, 0, 2, 3)
            .reshape(128, ktiles * 9 * M)
        ).astype(np.float32)

    # can the e-channels ride in branch3's ragged tile matmuls?
    NE = 9 * rem9 if use_e else 0
    rem3 = M3 % 128
    n3full = M3 - rem3
    EA = 32 * ((NE + 31) // 32)
    combine = use_e and rem3 > 0 and EA + rem3 <= 128

    arrs = {
        "W31d": np.ascontiguousarray(
            W31.T.reshape(4, 128, 2, 128).transpose(1, 2, 0, 3).reshape(128, 2 * 4 * 128)
        ).astype(np.float32)
    }
    if M0F:
        arrs["W0d"] = pack_ktpm(W0F, 4)
    if use_e:
        arrs["seld"] = np.ascontiguousarray(sel.reshape(128, 9 * rem9))
        if not combine:
            arrs["Ewd"] = pack_kpm(Ew, 4)
    if combine:
        Wride = np.zeros((EA + rem3, C), np.float32)
        Wride[:NE] = Ew
        Wride[EA:] = Wa1[n3full:]
        arrs["Wrided"] = pack_kpm(Wride, 4)
    if M1:
        arrs["W1d"] = pack_kpm(W1, 4)
    if M2:
        arrs["W33d"] = pack_ktpm(W33, 2)
    if (n3full if combine else M3):
        arrs["Wa1d"] = pack_kpm(Wa1[: n3full if combine else M3], 4)

    cols, ncol = _par_cols(counts)
    par = np.zeros((128, ncol), np.float32)
    par[:, cols["s31"]], par[:, cols["s31"] + 1] = s31[:128], s31[128:]
    par[:, cols["t31"]], par[:, cols["t31"] + 1] = t31[:128], t31[128:]
    for o, (mo, mw) in enumerate(_tiles_of(M3)):
        par[:mw, cols["a_s1"] + o] = a_s1[mo : mo + mw]
        par[:mw, cols["a_t1"] + o] = a_t1[mo : mo + mw]
        par[:mw, cols["pscale"] + o] = pscale[mo : mo + mw]
        par[:mw, cols["pbias"] + o] = pbias[mo : mo + mw]
    for o, (mo, mw) in enumerate(_tiles_of(M2)):
        par[:mw, cols["bias2"] + o] = bias2[mo : mo + mw]
    arrs["par"] = par
    return arrs, counts, jperm


# ---------------- device graph ----------------

def _build(counts):
    M0, M1, M2, M3 = counts
    cols, ncol = _par_cols(counts)
    M0F, rem9 = _e_split(M0)
    NE = 9 * rem9  # e-channel count
    rem3 = M3 % 128
    n3full = M3 - rem3
    EA = 32 * ((NE + 31) // 32)
    combine = rem9 > 0 and rem3 > 0 and EA + rem3 <= 128
    M3W = n3full if combine else M3  # channels served by the plain wa1 tiles
    n3tiles = _tiles_of(M3)
    nc = bacc.Bacc("TRN2", target_bir_lowering=False, debug=False, num_devices=NCORES)

    x_ext = nc.dram_tensor("x", [BL, C, H, W], F32R, kind="ExternalInput")
    W0_ext = nc.dram_tensor("W0d", [128, 4 * 9 * M0F], F32R, kind="ExternalInput") if M0F else None
    Ew_ext = nc.dram_tensor("Ewd", [128, 4 * NE], F32R, kind="ExternalInput") if rem9 and not combine else None
    sel_ext = nc.dram_tensor("seld", [128, NE], F32R, kind="ExternalInput") if rem9 else None
    Wr_ext = nc.dram_tensor("Wrided", [128, 4 * (EA + rem3)], F32R, kind="ExternalInput") if combine else None
    W1_ext = nc.dram_tensor("W1d", [128, 4 * M1], F32R, kind="ExternalInput") if M1 else None
    W31_ext = nc.dram_tensor("W31d", [128, 4 * MID], F32R, kind="ExternalInput")
    W33_ext = nc.dram_tensor("W33d", [128, 2 * 9 * M2], F32R, kind="ExternalInput") if M2 else None
    Wa1_ext = nc.dram_tensor("Wa1d", [128, 4 * M3W], F32R, kind="ExternalInput") if M3W else None
    par_ext = nc.dram_tensor("par", [128, ncol], F32, kind="ExternalInput")
    out_ext = nc.dram_tensor("out", [BL, C, H, W], F32, kind="ExternalOutput")

    off1, off2, off3 = M0, M0 + M1, M0 + M1 + M2

    with tile.TileContext(nc) as tc:
        with (
            tc.tile_pool(name="wpool", bufs=1) as wpool,
            tc.tile_pool(name="acts", bufs=1) as acts,
            tc.tile_pool(name="stage", bufs=4) as stage,
            tc.tile_pool(name="xstage", bufs=10) as xstage,
            tc.tile_pool(name="slabstage", bufs=1) as slabstage,
            tc.tile_pool(name="ps", bufs=8, space="PSUM") as ps,
        ):
            # ---- persistent weights; only wt31 loads before slab-0's x ----
            part = wpool.tile([128, ncol], F32)
            wt31 = wpool.tile([128, 2, 4, 128], F32R)
            wta1 = wpool.tile([128, 4, M3W], F32R, name="wta1") if M3W else None
            wtr = wpool.tile([128, 4, EA + rem3], F32R, name="wtr") if combine else None
            wt0 = wpool.tile([128, 4, 9, M0F], F32R, name="wt0") if M0F else None
            wte = wpool.tile([128, 4, NE], F32R, name="wte") if (rem9 and not combine) else None
            selt = wpool.tile([128, 9, rem9], F32R, name="selt") if rem9 else None
            wt1 = wpool.tile([128, 4, M1], F32R, name="wt1") if M1 else None
            wt33 = wpool.tile([128, 2, 9, M2], F32R, name="wt33") if M2 else None
            w31ap = W31_ext.ap().rearrange("p (o k m) -> p o k m", o=2, k=4)
            nc.sync.dma_start(wt31[:, 0], w31ap[:, 0])
            nc.sync.dma_start(wt31[:, 1], w31ap[:, 1])
            nc.gpsimd.dma_start(part[:], par_ext.ap())

            # small weights ride the gpsimd SWDGE lane, available early
            if combine:
                nc.gpsimd.dma_start(wtr[:], Wr_ext.ap().rearrange("p (k m) -> p k m", k=4))
            if M3W:
                nc.gpsimd.dma_start(wta1[:], Wa1_ext.ap().rearrange("p (k m) -> p k m", k=4))
            if rem9 and not combine:
                nc.gpsimd.dma_start(wte[:], Ew_ext.ap().rearrange("p (k m) -> p k m", k=4))
            if rem9:
                nc.gpsimd.dma_start(selt[:], sel_ext.ap().rearrange("p (t m) -> p t m", t=9))
            if M1:
                nc.gpsimd.dma_start(wt1[:], W1_ext.ap().rearrange("p (k m) -> p k m", k=4))

            def load_big_weights():
                # emitted after slab-0's x pieces so the scalar ring serves
                # the first chunks before streaming the large conv weights
                if M0F:
                    nc.scalar.dma_start(wt0[:], W0_ext.ap().rearrange("p (k t m) -> p k t m", k=4, t=9))
                if M2:
                    nc.scalar.dma_start(wt33[:], W33_ext.ap().rearrange("p (k t m) -> p k t m", k=2, t=9))

            # ---- persistent activation buffers (2 slab slots each) ----
            xt = [[acts.tile([128, BUF], F32R, name=f"xt{k}{s}", tag=f"x{k}s{s}") for s in range(2)] for k in range(4)]
            z1 = [[acts.tile([128, BUF], F32R, name=f"z1{k}{s}", tag=f"z{k}s{s}") for s in range(2)] for k in range(2)]
            za = [[acts.tile([128, BUF], F32, name=f"za{o}{s}", tag=f"za{o}s{s}") for s in range(2)] for o in range(len(n3tiles))]
            ze = [acts.tile([128, BUF], F32R, name=f"ze{s}", tag=f"zes{s}") for s in range(2)] if rem9 else None
            hs = [acts.tile([128, BUF], F32, name="hs0", tag="hs0")] * max(1, len(n3tiles))

            # zero only the pad regions (row pads, col pads, slack), not the
            # whole buffers: three tiny memsets per buffer, split over engines.
            def pad_memsets(t, eng):
                a = t[:].bitcast(mybir.dt.uint32)
                eng.memset(a[:, 0:59], 0)  # slack + row 0
                # col pads: w in {0,57} of every row == flat {58r, 58r+1}
                eng.memset(a[:, 0 : 30 * WP].rearrange("p (r w) -> p r w", w=WP)[:, :, 0:2], 0)
                eng.memset(a[:, 29 * WP + 1 : BUF], 0)  # row 29 + tail slack

            def emit_pad_memsets(sidx):
                for group in (xt, z1, za, [ze] if rem9 else []):
                    for pair in group:
                        pad_memsets(pair[sidx], nc.vector)

            emit_pad_memsets(0)  # slot-1 pads are zeroed during slab 0 (see loop)

            OUT_STARTS = [1, 8, 15, 22]  # slab-local output row starts (7 rows)

            def evict_to_out(acc, mw, segs, b, g0):
                """PSUM rows [0,mw) -> valid cols -> stage; then one DMA per
                (p_lo, p_hi, ch0) segment (PSUM reads must start at part 0)."""
                st = stage.tile([mw, 7 * 56], F32, name="st", tag="st")
                src = acc[0:mw].rearrange("p (r w) -> p r w", w=WP)[:, :, 1:57]
                dst = st[:].rearrange("p (r w) -> p r w", w=56)
                nc.scalar.activation(dst, src, AF.Copy)
                for (p_lo, p_hi, ch0) in segs:
                    nc.sync.dma_start(
                        out_ext.ap()[b, ch0 : ch0 + p_hi - p_lo, g0 : g0 + 7, :],
                        st[p_lo:p_hi].rearrange("p (r w) -> p r w", w=56),
                    )

            def emit_x(b, s):
                """DMA one x slab into compact staging (both HWDGE rings) and
                place into the padded layout on DVE/ACT."""
                x_pieces = [(1, 7), (8, 7), (15, 7), (22, 8)] if s == 0 else [(0, 7), (7, 7), (14, 7), (21, 8)]
                xs_tiles = {}
                for ci, (lp, pn) in enumerate(x_pieces):
                    for k in range(4):
                        xs = xstage.tile([128, 8 * 56], F32R, name="xs", tag="xs")
                        xs_tiles[(ci, k)] = xs
                        deng = nc.sync if k < 2 else nc.scalar
                        deng.dma_start(
                            xs[:, 0 : pn * 56],
                            x_ext.ap()[b, 128 * k : 128 * (k + 1),
                                       SLAB_ROWS * s + lp - 1 : SLAB_ROWS * s + lp - 1 + pn, :],
                        )
                return x_pieces, xs_tiles

            def emit_copies(b, s, x_pieces, xs_tiles):
                for ci, (lp, pn) in enumerate(x_pieces):
                    for k in range(4):
                        xs = xs_tiles[(ci, k)]
                        dst = xt[k][s][:, lp * WP + 2 : lp * WP + 2 + pn * WP].rearrange(
                            "p (r w) -> p r w", w=WP
                        )[:, :, 0:56]
                        srcv = xs[:, 0 : pn * 56].rearrange("p (r w) -> p r w", w=56)
                        if k % 2 == 0:
                            nc.vector.tensor_copy(dst, srcv)
                        else:
                            nc.scalar.activation(dst, srcv, AF.Copy)

            def emit_mids(b, s, xs_tiles):
                mid_chunks = [(1, 7), (8, 7), (15, 7), (22, 8)] if s == 0 else [(0, 7), (7, 7), (14, 7), (21, 8)]
                for ci, (l0c, nr) in enumerate(mid_chunks):
                    NV = nr * 56

                    def mid_conv(weight_ap, mw_, dstbuf, scale_c=None, bias_c=None):
                        acc = ps.tile([mw_, NV], F32, name="acc", tag="ps")
                        for k in range(4):
                            nc.tensor.matmul(
                                acc[:], weight_ap(k), xs_tiles[(ci, k)][:, 0:NV],
                                start=(k == 0), stop=(k == 3),
                            )
                        dst = dstbuf[:mw_, l0c * WP + 2 : l0c * WP + 2 + nr * WP].rearrange(
                            "p (r w) -> p r w", w=WP
                        )[:, :, 0:56]
                        srcv = acc[:].rearrange("p (r w) -> p r w", w=56)
                        if scale_c is None:
                            nc.scalar.activation(dst, srcv, AF.Copy)
                        else:
                            nc.scalar.activation(
                                dst, srcv, AF.Identity,
                                bias=part[:mw_, bias_c : bias_c + 1],
                                scale=part[:mw_, scale_c : scale_c + 1],
                            )

                    for o in range(2):
                        mid_conv(lambda k, o=o: wt31[:, o, k, :], 128, z1[o][s],
                                 cols["s31"] + o, cols["t31"] + o)
                    for o, (mo, mw) in enumerate(_tiles_of(M3W)):
                        mid_conv(lambda k, mo=mo, mw=mw: wta1[:, k, mo : mo + mw], mw,
                                 za[o][s], cols["a_s1"] + o, cols["a_t1"] + o)
                    if combine:
                        # one group computes the e-channels (rows 0:NE) and
                        # branch3's ragged tile (rows EA:EA+rem3, 32-aligned)
                        o = len(n3tiles) - 1
                        acc = ps.tile([EA + rem3, NV], F32, name="acc", tag="ps")
                        for k in range(4):
                            nc.tensor.matmul(
                                acc[:], wtr[:, k, :], xs_tiles[(ci, k)][:, 0:NV],
                                start=(k == 0), stop=(k == 3),
                            )
                        dste = ze[s][:NE, l0c * WP + 2 : l0c * WP + 2 + nr * WP].rearrange(
                            "p (r w) -> p r w", w=WP
                        )[:, :, 0:56]
                        nc.scalar.activation(
                            dste, acc[0:NE].rearrange("p (r w) -> p r w", w=56), AF.Copy
                        )
                        dsta = za[o][s][:rem3, l0c * WP + 2 : l0c * WP + 2 + nr * WP].rearrange(
                            "p (r w) -> p r w", w=WP
                        )[:, :, 0:56]
                        nc.scalar.activation(
                            dsta, acc[EA : EA + rem3].rearrange("p (r w) -> p r w", w=56),
                            AF.Identity,
                            bias=part[:rem3, cols["a_t1"] + o : cols["a_t1"] + o + 1],
                            scale=part[:rem3, cols["a_s1"] + o : cols["a_s1"] + o + 1],
                        )
                    elif rem9:
                        mid_conv(lambda k: wte[:, k, :], NE, ze[s])

            def emit_pool(b, s):
                add = mybir.AluOpType.add
                for o, (mo, mw) in enumerate(n3tiles):
                    zz = za[o][s]
                    hh = hs[o]
                    nc.vector.tensor_tensor(
                        hh[:mw, 1 : BUF - 1], zz[:mw, 0 : BUF - 2], zz[:mw, 2:BUF], add
                    )
                    nc.vector.tensor_tensor(
                        hh[:mw, 1 : BUF - 1], hh[:mw, 1 : BUF - 1], zz[:mw, 1 : BUF - 1], add
                    )

                    def v3(buf, base, mwl=mw):
                        return buf[:mwl, base : base + 28 * WP].rearrange(
                            "p (r w) -> p r w", w=WP
                        )[:, :, 0:56]

                    nc.vector.tensor_tensor(v3(zz, 60), v3(hh, 2), v3(hh, 118), add)
                    nc.vector.tensor_tensor(v3(zz, 60), v3(zz, 60), v3(hh, 60), add)
                    st = slabstage.tile([mw, 28 * 56], F32, name="sst", tag="sst")
                    nc.scalar.activation(
                        st[:].rearrange("p (r w) -> p r w", w=56),
                        v3(zz, 60), AF.Identity,
                        bias=part[:mw, cols["pbias"] + o : cols["pbias"] + o + 1],
                        scale=part[:mw, cols["pscale"] + o : cols["pscale"] + o + 1],
                    )
                    g0 = SLAB_ROWS * s
                    nc.sync.dma_start(
                        out_ext.ap()[b, off3 + mo : off3 + mo + mw, g0 : g0 + 28, :],
                        st[:].rearrange("p (r w) -> p r w", w=56),
                    )

            def emit_b0F(b, s, l0c):
                q0 = l0c * WP + 1
                N = 7 * WP
                g0 = l0c - 1 + SLAB_ROWS * s
                for (mo, mw) in _tiles_of(M0F):
                    acc = ps.tile([mw, N], F32, name="acc", tag="ps")
                    first = True
                    for t in CENTER_FIRST:
                        dh, dw = t // 3, t % 3
                        qr = q0 + (dh - 1) * WP + (dw - 1)
                        for k in range(4):
                            nc.tensor.matmul(
                                acc[:], wt0[:, k, t, mo : mo + mw],
                                xt[k][s][:, qr : qr + N],
                                start=first, stop=(t == 8 and k == 3),
                            )
                            first = False
                    evict_to_out(acc, mw, [(0, mw, mo)], b, g0)

            def emit_tail(b, s, l0c):
                if not rem9:
                    return
                q0 = l0c * WP + 1
                N = 7 * WP
                g0 = l0c - 1 + SLAB_ROWS * s
                acc = ps.tile([rem9, N], F32, name="acc", tag="ps")
                first = True
                for t in CENTER_FIRST:
                    dh, dw = t // 3, t % 3
                    qr = q0 + (dh - 1) * WP + (dw - 1)
                    nc.tensor.matmul(
                        acc[:], selt[:NE, t, :], ze[s][:NE, qr : qr + N],
                        start=first, stop=(t == 8),
                    )
                    first = False
                evict_to_out(acc, rem9, [(0, rem9, M0F)], b, g0)

            def emit_b1(b, s, l0c):
                q0 = l0c * WP + 1
                N = 7 * WP
                g0 = l0c - 1 + SLAB_ROWS * s
                for (mo, mw) in _tiles_of(M1):
                    acc = ps.tile([mw, N], F32, name="acc", tag="ps")
                    for k in range(4):
                        nc.tensor.matmul(
                            acc[:], wt1[:, k, mo : mo + mw], xt[k][s][:, q0 : q0 + N],
                            start=(k == 0), stop=(k == 3),
                        )
                    evict_to_out(acc, mw, [(0, mw, off1 + mo)], b, g0)

            def emit_b2c2(b, s, l0c):
                q0 = l0c * WP + 1
                N = 7 * WP
                g0 = l0c - 1 + SLAB_ROWS * s
                for o, (mo, mw) in enumerate(_tiles_of(M2)):
                    acc = ps.tile([mw, N], F32, name="acc", tag="ps")
                    first = True
                    for t in CENTER_FIRST:
                        dh, dw = t // 3, t % 3
                        qr = q0 + (dh - 1) * WP + (dw - 1)
                        for k in range(2):
                            nc.tensor.matmul(
                                acc[:], wt33[:, k, t, mo : mo + mw],
                                z1[k][s][:, qr : qr + N],
                                start=first, stop=(t == 8 and k == 1),
                            )
                            first = False
                    st = stage.tile([mw, 7 * 56], F32, name="st", tag="st")
                    nc.scalar.activation(
                        st[:].rearrange("p (r w) -> p r w", w=56),
                        acc[:].rearrange("p (r w) -> p r w", w=WP)[:, :, 1:57],
                        AF.Identity,
                        bias=part[:mw, cols["bias2"] + o : cols["bias2"] + o + 1],
                    )
                    nc.sync.dma_start(
                        out_ext.ap()[b, off2 + mo : off2 + mo + mw, g0 : g0 + 7, :],
                        st[:].rearrange("p (r w) -> p r w", w=56),
                    )

            # ---- pipelined emission schedule ----
            slabs = [(b, s) for b in range(BL) for s in range(2)]
            pieces, tiles = emit_x(*slabs[0])
            xq = {0: (pieces, tiles)}
            for idx, (b, s) in enumerate(slabs):
                x_pieces, xs_tiles = xq.pop(idx)
                if idx == 0:
                    load_big_weights()
                if idx == 1:
                    emit_pad_memsets(1)
                emit_copies(b, s, x_pieces, xs_tiles)
                emit_mids(b, s, xs_tiles)
                emit_pool(b, s)
                if idx + 1 < len(slabs):
                    xq[idx + 1] = emit_x(*slabs[idx + 1])
                if idx == 0:
                    # slab 0: run small-weight branches first so the large
                    # conv weights have time to stream in behind the x pieces
                    for l0c in OUT_STARTS:
                        emit_b1(b, s, l0c)
                    for l0c in OUT_STARTS:
                        emit_tail(b, s, l0c)
                    for l0c in OUT_STARTS:
                        emit_b2c2(b, s, l0c)
                    for l0c in OUT_STARTS:
                        emit_b0F(b, s, l0c)
                else:
                    for l0c in OUT_STARTS:
                        emit_b0F(b, s, l0c)
                        emit_tail(b, s, l0c)
                        emit_b1(b, s, l0c)
                    for l0c in OUT_STARTS:
                        emit_b2c2(b, s, l0c)

    nc.compile()
    return nc


# ---------------- entry point ----------------

def kernel(x, w_main, w_1x1, w31, bn31, w33, bn33, wa1, bna1, bna2, fuse_weight, c_score):
    global LAST_EXEC_NS
    arrs, counts, jperm = _prep(
        w_main, w_1x1, w31, bn31, w33, bn33, wa1, bna1, bna2, fuse_weight, c_score
    )
    if counts not in _CACHE:
        _CACHE[counts] = _build(counts)
    nc = _CACHE[counts]

    x = np.ascontiguousarray(x, dtype=np.float32)
    in_maps = []
    for i in range(NCORES):
        m = {"x": np.ascontiguousarray(x[BL * i : BL * (i + 1)])}
        m.update(arrs)
        in_maps.append(m)

    res = run_bass_kernel_spmd(nc, in_maps, list(range(NCORES)), trace=PROFILE)
    LAST_EXEC_NS = res.exec_time_ns

    full = np.empty((B, C, H, W), np.float32)
    for i in range(NCORES):
        full[BL * i : BL * (i + 1)] = res.results[i]["out"]
    out = np.empty_like(full)
    out[:, jperm] = full
    return out


# revision 33
# speedup vs baseline: 1.3142x; 1.1299x over previous
"""Trainium2 Bass kernel for nn_ACS (4-branch conv block + top-k channel select).

Strategy:
- Host: top-k of c_score; gather ONLY the 512 surviving output channels;
  fold fuse_weight, all BN affines and the sigmoid scale into conv
  weights / per-channel eviction affines. Output channel permutation is
  applied host-side after gather (free).
- Device (SPMD, 8 cores, 2 images each; no collectives):
  * activations live in a zero-padded [C, 58*58]-style layout; 3x3 convs
    run as 9 shifted matmuls accumulating in PSUM (fp32r: full-rate
    fp32-rounded matmul dtype).
  * branch0 = 3x3 conv; branch1 = 1x1 conv; branch2 = 1x1 conv ->
    BN-affine eviction -> 3x3 conv (+bias evict); branch3 = 1x1 conv ->
    BN-affine eviction -> separable 3x3 sum-pool on VectorE -> affine evict.
  * branch0's ragged tail tile is topped up with branch1 channels
    (center-tap only) so the 128-wide PE columns stay productive.
  * per-image work is split in two 28-row slabs, chunked into 7/8-row
    matmul groups (free dim 406/464 <= one PSUM bank).
"""

import numpy as np

import concourse.mybir as mybir
import concourse.tile as tile
from concourse import bacc
from concourse.bass_utils import run_bass_kernel_spmd

# ---- problem constants (hardcoded per spec) ----
B, C, H, W = 16, 512, 56, 56
MID = 256
NCORES = 8
BL = B // NCORES  # images per core
WP = H + 2  # padded row width 58
SLAB_ROWS = 28
BUF = 30 * WP + 4  # slab buffer free size (30 padded rows + slack) = 1744
EPS = 1e-5
CENTER_FIRST = [4, 0, 1, 2, 3, 5, 6, 7, 8]  # tap order: center tap starts PSUM

F32 = mybir.dt.float32
F32R = mybir.dt.float32r
AF = mybir.ActivationFunctionType

PROFILE = False
LAST_EXEC_NS = None
_CACHE = {}


def _tiles_of(M):
    out = []
    o = 0
    while o < M:
        out.append((o, min(128, M - o)))
        o += 128
    return out


def _e_split(M0):
    """b0 tail: use tap-expansion when the ragged remainder is small."""
    rem9 = M0 % 128
    use_e = 0 < rem9 <= 28
    return (M0 - rem9, rem9) if use_e else (M0, 0)


def _par_cols(counts):
    M0, M1, M2, M3 = counts
    n2, n3 = len(_tiles_of(M2)), len(_tiles_of(M3))
    cols = {"s31": 0, "t31": 2}
    c = 4
    cols["a_s1"] = c
    c += n3
    cols["a_t1"] = c
    c += n3
    cols["bias2"] = c
    c += n2
    cols["pscale"] = c
    c += n3
    cols["pbias"] = c
    c += n3
    return cols, c


# ---------------- host-side folding ----------------

def _bn_fold(p):
    g, b, m, v = [p[i].astype(np.float64) for i in range(4)]
    s = g / np.sqrt(v + EPS)
    t = b - m * s
    return s.astype(np.float32), t.astype(np.float32)


def _prep(w_main, w_1x1, w31, bn31, w33, bn33, wa1, bna1, bna2, fuse_weight, c_score):
    f = [float(fuse_weight[i]) for i in range(4)]
    s31, t31 = _bn_fold(bn31)
    s33, t33 = _bn_fold(bn33)
    sa1, ta1 = _bn_fold(bna1)
    sa2, ta2 = _bn_fold(bna2)

    ind = np.argsort(-c_score, kind="stable")[:C]
    scale = (1.0 / (1.0 + np.exp(-c_score.astype(np.float64))))[ind].astype(np.float32)

    js = {b_: [] for b_ in range(4)}
    cs = {b_: [] for b_ in range(4)}
    for j, gi in enumerate(ind):
        b_ = int(gi) // 256
        js[b_].append(j)
        cs[b_].append(int(gi) % 256)
    c0, c1, c2, c3 = [np.array(cs[i], np.int64) for i in range(4)]
    sc = [scale[np.array(js[i], np.int64)] if js[i] else np.zeros(0, np.float32) for i in range(4)]

    W0 = w_main[c0] * f[0] * sc[0][:, None, None, None]            # [M0,C,3,3]
    W1 = w_1x1[c1, :, 0, 0] * f[1] * sc[1][:, None]                # [M1,C]
    W31 = w31[:, :, 0, 0] * f[2]                                   # [MID,C]
    W33 = w33[c2] * (s33[c2] * sc[2])[:, None, None, None]         # [M2,MID,3,3]
    bias2 = t33[c2] * sc[2]
    Wa1 = wa1[c3, :, 0, 0] * f[3]                                  # [M3,C]
    a_s1, a_t1 = sa1[c3], ta1[c3]
    pscale = sa2[c3] * sc[3] / 9.0
    pbias = ta2[c3] * sc[3]

    jperm = np.array(js[0] + js[1] + js[2] + js[3], dtype=np.int64)
    counts = (len(c0), len(c1), len(c2), len(c3))
    M0, M1, M2, M3 = counts

    # branch0: full 128-wide tiles; small ragged tail handled via tap
    # expansion (e-channels) instead of a nearly-empty 9-tap tile
    rem9 = M0 % 128
    use_e = 0 < rem9 <= 28
    M0F = M0 - rem9 if use_e else M0
    W0F = W0[:M0F]
    if use_e:
        W0T = W0[M0F:]  # [rem9, C, 3, 3]
        # e-channel 1x1 weights: channel (t*rem9 + j) = tap t of tail ch j
        Ew = np.ascontiguousarray(
            W0T.transpose(2, 3, 0, 1).reshape(9 * rem9, C)
        )
        # selector for the tail conv: lhsT[i, t, j] = 1 iff i == t*rem9+j
        sel = np.zeros((128, 9, rem9), np.float32)
        for t_ in range(9):
            for j_ in range(rem9):
                sel[t_ * rem9 + j_, t_, j_] = 1.0

    def pack_kpm(Wmat, ktiles):
        M = Wmat.shape[0]
        return np.ascontiguousarray(
            Wmat.T.reshape(ktiles, 128, M).transpose(1, 0, 2).reshape(128, ktiles * M)
        ).astype(np.float32)

    def pack_ktpm(Wconv, ktiles):
        M = Wconv.shape[0]
        return np.ascontiguousarray(
            Wconv.transpose(1, 2, 3, 0)
            .reshape(ktiles, 128, 9, M)
            .transpose(1, 0, 2, 3)
            .reshape(128, ktiles * 9 * M)
        ).astype(np.float32)

    # can the e-channels ride in branch3's ragged tile matmuls?
    NE = 9 * rem9 if use_e else 0
    rem3 = M3 % 128
    n3full = M3 - rem3
    EA = 32 * ((NE + 31) // 32)
    combine = use_e and rem3 > 0 and EA + rem3 <= 128

    arrs = {
        "W31d": np.ascontiguousarray(
            W31.T.reshape(4, 128, 2, 128).transpose(1, 2, 0, 3).reshape(128, 2 * 4 * 128)
        ).astype(np.float32)
    }
    if M0F:
        arrs["W0d"] = pack_ktpm(W0F, 4)
    if use_e:
        arrs["seld"] = np.ascontiguousarray(sel.reshape(128, 9 * rem9))
        if not combine:
            arrs["Ewd"] = pack_kpm(Ew, 4)
    if combine:
        Wride = np.zeros((EA + rem3, C), np.float32)
        Wride[:NE] = Ew
        Wride[EA:] = Wa1[n3full:]
        arrs["Wrided"] = pack_kpm(Wride, 4)
    if M1:
        arrs["W1d"] = pack_kpm(W1, 4)
    if M2:
        arrs["W33d"] = pack_ktpm(W33, 2)
    if (n3full if combine else M3):
        arrs["Wa1d"] = pack_kpm(Wa1[: n3full if combine else M3], 4)

    cols, ncol = _par_cols(counts)
    par = np.zeros((128, ncol), np.float32)
    par[:, cols["s31"]], par[:, cols["s31"] + 1] = s31[:128], s31[128:]
    par[:, cols["t31"]], par[:, cols["t31"] + 1] = t31[:128], t31[128:]
    for o, (mo, mw) in enumerate(_tiles_of(M3)):
        par[:mw, cols["a_s1"] + o] = a_s1[mo : mo + mw]
        par[:mw, cols["a_t1"] + o] = a_t1[mo : mo + mw]
        par[:mw, cols["pscale"] + o] = pscale[mo : mo + mw]
        par[:mw, cols["pbias"] + o] = pbias[mo : mo + mw]
    for o, (mo, mw) in enumerate(_tiles_of(M2)):
        par[:mw, cols["bias2"] + o] = bias2[mo : mo + mw]
    arrs["par"] = par
    return arrs, counts, jperm


# ---------------- device graph ----------------

def _build(counts):
    M0, M1, M2, M3 = counts
    cols, ncol = _par_cols(counts)
    M0F, rem9 = _e_split(M0)
    NE = 9 * rem9  # e-channel count
    rem3 = M3 % 128
    n3full = M3 - rem3
    EA = 32 * ((NE + 31) // 32)
    combine = rem9 > 0 and rem3 > 0 and EA + rem3 <= 128
    M3W = n3full if combine else M3  # channels served by the plain wa1 tiles
    n3tiles = _tiles_of(M3)
    nc = bacc.Bacc("TRN2", target_bir_lowering=False, debug=False, num_devices=NCORES)

    x_ext = nc.dram_tensor("x", [BL, C, H, W], F32R, kind="ExternalInput")
    W0_ext = nc.dram_tensor("W0d", [128, 4 * 9 * M0F], F32R, kind="ExternalInput") if M0F else None
    Ew_ext = nc.dram_tensor("Ewd", [128, 4 * NE], F32R, kind="ExternalInput") if rem9 and not combine else None
    sel_ext = nc.dram_tensor("seld", [128, NE], F32R, kind="ExternalInput") if rem9 else None
    Wr_ext = nc.dram_tensor("Wrided", [128, 4 * (EA + rem3)], F32R, kind="ExternalInput") if combine else None
    W1_ext = nc.dram_tensor("W1d", [128, 4 * M1], F32R, kind="ExternalInput") if M1 else None
    W31_ext = nc.dram_tensor("W31d", [128, 4 * MID], F32R, kind="ExternalInput")
    W33_ext = nc.dram_tensor("W33d", [128, 2 * 9 * M2], F32R, kind="ExternalInput") if M2 else None
    Wa1_ext = nc.dram_tensor("Wa1d", [128, 4 * M3W], F32R, kind="ExternalInput") if M3W else None
    par_ext = nc.dram_tensor("par", [128, ncol], F32, kind="ExternalInput")
    out_ext = nc.dram_tensor("out", [BL, C, H, W], F32, kind="ExternalOutput")

    off1, off2, off3 = M0, M0 + M1, M0 + M1 + M2

    with tile.TileContext(nc) as tc:
        with (
            tc.tile_pool(name="wpool", bufs=1) as wpool,
            tc.tile_pool(name="acts", bufs=1) as acts,
            tc.tile_pool(name="stage", bufs=4) as stage,
            tc.tile_pool(name="xstage", bufs=10) as xstage,
            tc.tile_pool(name="slabstage", bufs=1) as slabstage,
            tc.tile_pool(name="ps", bufs=8, space="PSUM") as ps,
        ):
            # ---- persistent weights; only wt31 loads before slab-0's x ----
            part = wpool.tile([128, ncol], F32)
            wt31 = wpool.tile([128, 2, 4, 128], F32R)
            wta1 = wpool.tile([128, 4, M3W], F32R, name="wta1") if M3W else None
            wtr = wpool.tile([128, 4, EA + rem3], F32R, name="wtr") if combine else None
            wt0 = wpool.tile([128, 4, 9, M0F], F32R, name="wt0") if M0F else None
            wte = wpool.tile([128, 4, NE], F32R, name="wte") if (rem9 and not combine) else None
            selt = wpool.tile([128, 9, rem9], F32R, name="selt") if rem9 else None
            wt1 = wpool.tile([128, 4, M1], F32R, name="wt1") if M1 else None
            wt33 = wpool.tile([128, 2, 9, M2], F32R, name="wt33") if M2 else None
            w31ap = W31_ext.ap().rearrange("p (o k m) -> p o k m", o=2, k=4)
            nc.sync.dma_start(wt31[:, 0], w31ap[:, 0])
            nc.sync.dma_start(wt31[:, 1], w31ap[:, 1])
            nc.gpsimd.dma_start(part[:], par_ext.ap())

            # small weights ride the gpsimd SWDGE lane, available early
            if combine:
                nc.gpsimd.dma_start(wtr[:], Wr_ext.ap().rearrange("p (k m) -> p k m", k=4))
            if M3W:
                nc.gpsimd.dma_start(wta1[:], Wa1_ext.ap().rearrange("p (k m) -> p k m", k=4))
            if rem9 and not combine:
                nc.gpsimd.dma_start(wte[:], Ew_ext.ap().rearrange("p (k m) -> p k m", k=4))
            if rem9:
                nc.gpsimd.dma_start(selt[:], sel_ext.ap().rearrange("p (t m) -> p t m", t=9))
            if M1:
                nc.gpsimd.dma_start(wt1[:], W1_ext.ap().rearrange("p (k m) -> p k m", k=4))

            def load_big_weights():
                # emitted after slab-0's x pieces so the scalar ring serves
                # the first chunks before streaming the large conv weights
                if M0F:
                    nc.scalar.dma_start(wt0[:], W0_ext.ap().rearrange("p (k t m) -> p k t m", k=4, t=9))
                if M2:
                    nc.scalar.dma_start(wt33[:], W33_ext.ap().rearrange("p (k t m) -> p k t m", k=2, t=9))

            # ---- persistent activation buffers (2 slab slots each) ----
            xt = [[acts.tile([128, BUF], F32R, name=f"xt{k}{s}", tag=f"x{k}s{s}") for s in range(2)] for k in range(4)]
            z1 = [[acts.tile([128, BUF], F32R, name=f"z1{k}{s}", tag=f"z{k}s{s}") for s in range(2)] for k in range(2)]
            za = [[acts.tile([128, BUF], F32, name=f"za{o}{s}", tag=f"za{o}s{s}") for s in range(2)] for o in range(len(n3tiles))]
            ze = [acts.tile([128, BUF], F32R, name=f"ze{s}", tag=f"zes{s}") for s in range(2)] if rem9 else None
            hs = [acts.tile([128, BUF], F32, name="hs0", tag="hs0")] * max(1, len(n3tiles))

            # zero only the pad regions (row pads, col pads, slack), not the
            # whole buffers: three tiny memsets per buffer, split over engines.
            def pad_memsets(t, eng):
                a = t[:].bitcast(mybir.dt.uint32)
                eng.memset(a[:, 0:59], 0)  # slack + row 0
                # col pads: w in {0,57} of every row == flat {58r, 58r+1}
                eng.memset(a[:, 0 : 30 * WP].rearrange("p (r w) -> p r w", w=WP)[:, :, 0:2], 0)
                eng.memset(a[:, 29 * WP + 1 : BUF], 0)  # row 29 + tail slack

            def emit_pad_memsets(sidx):
                for group in (xt, z1, za, [ze] if rem9 else []):
                    for pair in group:
                        pad_memsets(pair[sidx], nc.vector)

            emit_pad_memsets(0)  # slot-1 pads are zeroed during slab 0 (see loop)

            OUT_STARTS = [1, 8, 15, 22]  # slab-local output row starts (7 rows)

            def xwin(buf, mwl, l0c, dh, dw):
                """7-row valid-column window of a padded buffer for tap (dh, dw)."""
                base = (l0c + dh - 1) * WP + dw + 1
                return buf[:mwl, base : base + 7 * WP].rearrange("p (r w) -> p r w", w=WP)[:, :, 0:56]

            def evict_to_out(acc, mw, segs, b, g0):
                """compact PSUM [mw, 7*56] -> stage; then one DMA per
                (p_lo, p_hi, ch0) segment (PSUM reads must start at part 0)."""
                st = stage.tile([mw, 7 * 56], F32, name="st", tag="st")
                nc.scalar.activation(st[:], acc[0:mw], AF.Copy)
                for (p_lo, p_hi, ch0) in segs:
                    nc.sync.dma_start(
                        out_ext.ap()[b, ch0 : ch0 + p_hi - p_lo, g0 : g0 + 7, :],
                        st[p_lo:p_hi].rearrange("p (r w) -> p r w", w=56),
                    )

            def emit_x(b, s):
                """DMA one x slab into compact staging (both HWDGE rings) and
                place into the padded layout on DVE/ACT."""
                x_pieces = [(1, 7), (8, 7), (15, 7), (22, 8)] if s == 0 else [(0, 7), (7, 7), (14, 7), (21, 8)]
                xs_tiles = {}
                for ci, (lp, pn) in enumerate(x_pieces):
                    for k in range(4):
                        xs = xstage.tile([128, 8 * 56], F32R, name="xs", tag="xs")
                        xs_tiles[(ci, k)] = xs
                        deng = nc.sync if k < 2 else nc.scalar
                        deng.dma_start(
                            xs[:, 0 : pn * 56],
                            x_ext.ap()[b, 128 * k : 128 * (k + 1),
                                       SLAB_ROWS * s + lp - 1 : SLAB_ROWS * s + lp - 1 + pn, :],
                        )
                return x_pieces, xs_tiles

            def emit_copies(b, s, x_pieces, xs_tiles):
                for ci, (lp, pn) in enumerate(x_pieces):
                    for k in range(4):
                        xs = xs_tiles[(ci, k)]
                        dst = xt[k][s][:, lp * WP + 2 : lp * WP + 2 + pn * WP].rearrange(
                            "p (r w) -> p r w", w=WP
                        )[:, :, 0:56]
                        srcv = xs[:, 0 : pn * 56].rearrange("p (r w) -> p r w", w=56)
                        if k % 2 == 0:
                            nc.vector.tensor_copy(dst, srcv)
                        else:
                            nc.scalar.activation(dst, srcv, AF.Copy)

            def emit_mids(b, s, xs_tiles):
                mid_chunks = [(1, 7), (8, 7), (15, 7), (22, 8)] if s == 0 else [(0, 7), (7, 7), (14, 7), (21, 8)]
                for ci, (l0c, nr) in enumerate(mid_chunks):
                    NV = nr * 56

                    def mid_conv(weight_ap, mw_, dstbuf, scale_c=None, bias_c=None):
                        acc = ps.tile([mw_, NV], F32, name="acc", tag="ps")
                        for k in range(4):
                            nc.tensor.matmul(
                                acc[:], weight_ap(k), xs_tiles[(ci, k)][:, 0:NV],
                                start=(k == 0), stop=(k == 3),
                            )
                        dst = dstbuf[:mw_, l0c * WP + 2 : l0c * WP + 2 + nr * WP].rearrange(
                            "p (r w) -> p r w", w=WP
                        )[:, :, 0:56]
                        srcv = acc[:].rearrange("p (r w) -> p r w", w=56)
                        if scale_c is None:
                            nc.scalar.activation(dst, srcv, AF.Copy)
                        else:
                            nc.scalar.activation(
                                dst, srcv, AF.Identity,
                                bias=part[:mw_, bias_c : bias_c + 1],
                                scale=part[:mw_, scale_c : scale_c + 1],
                            )

                    for o in range(2):
                        mid_conv(lambda k, o=o: wt31[:, o, k, :], 128, z1[o][s],
                                 cols["s31"] + o, cols["t31"] + o)
                    for o, (mo, mw) in enumerate(_tiles_of(M3W)):
                        mid_conv(lambda k, mo=mo, mw=mw: wta1[:, k, mo : mo + mw], mw,
                                 za[o][s], cols["a_s1"] + o, cols["a_t1"] + o)
                    if combine:
                        # one group computes the e-channels (rows 0:NE) and
                        # branch3's ragged tile (rows EA:EA+rem3, 32-aligned)
                        o = len(n3tiles) - 1
                        acc = ps.tile([EA + rem3, NV], F32, name="acc", tag="ps")
                        for k in range(4):
                            nc.tensor.matmul(
                                acc[:], wtr[:, k, :], xs_tiles[(ci, k)][:, 0:NV],
                                start=(k == 0), stop=(k == 3),
                            )
                        dste = ze[s][:NE, l0c * WP + 2 : l0c * WP + 2 + nr * WP].rearrange(
                            "p (r w) -> p r w", w=WP
                        )[:, :, 0:56]
                        nc.scalar.activation(
                            dste, acc[0:NE].rearrange("p (r w) -> p r w", w=56), AF.Copy
                        )
                        dsta = za[o][s][:rem3, l0c * WP + 2 : l0c * WP + 2 + nr * WP].rearrange(
                            "p (r w) -> p r w", w=WP
                        )[:, :, 0:56]
                        nc.scalar.activation(
                            dsta, acc[EA : EA + rem3].rearrange("p (r w) -> p r w", w=56),
                            AF.Identity,
                            bias=part[:rem3, cols["a_t1"] + o : cols["a_t1"] + o + 1],
                            scale=part[:rem3, cols["a_s1"] + o : cols["a_s1"] + o + 1],
                        )
                    elif rem9:
                        mid_conv(lambda k: wte[:, k, :], NE, ze[s])

            def emit_pool(b, s):
                add = mybir.AluOpType.add
                for o, (mo, mw) in enumerate(n3tiles):
                    zz = za[o][s]
                    hh = hs[o]
                    nc.vector.tensor_tensor(
                        hh[:mw, 1 : BUF - 1], zz[:mw, 0 : BUF - 2], zz[:mw, 2:BUF], add
                    )
                    nc.vector.tensor_tensor(
                        hh[:mw, 1 : BUF - 1], hh[:mw, 1 : BUF - 1], zz[:mw, 1 : BUF - 1], add
                    )

                    def v3(buf, base, mwl=mw):
                        return buf[:mwl, base : base + 28 * WP].rearrange(
                            "p (r w) -> p r w", w=WP
                        )[:, :, 0:56]

                    nc.vector.tensor_tensor(v3(zz, 60), v3(hh, 2), v3(hh, 118), add)
                    nc.vector.tensor_tensor(v3(zz, 60), v3(zz, 60), v3(hh, 60), add)
                    st = slabstage.tile([mw, 28 * 56], F32, name="sst", tag="sst")
                    nc.scalar.activation(
                        st[:].rearrange("p (r w) -> p r w", w=56),
                        v3(zz, 60), AF.Identity,
                        bias=part[:mw, cols["pbias"] + o : cols["pbias"] + o + 1],
                        scale=part[:mw, cols["pscale"] + o : cols["pscale"] + o + 1],
                    )
                    g0 = SLAB_ROWS * s
                    nc.sync.dma_start(
                        out_ext.ap()[b, off3 + mo : off3 + mo + mw, g0 : g0 + 28, :],
                        st[:].rearrange("p (r w) -> p r w", w=56),
                    )

            def emit_b0F(b, s, l0c):
                g0 = l0c - 1 + SLAB_ROWS * s
                for (mo, mw) in _tiles_of(M0F):
                    acc = ps.tile([mw, 7 * 56], F32, name="acc", tag="ps")
                    accv = acc[:].rearrange("p (r w) -> p r w", w=56)
                    first = True
                    for t in CENTER_FIRST:
                        dh, dw = t // 3, t % 3
                        for k in range(4):
                            nc.tensor.matmul(
                                accv, wt0[:, k, t, mo : mo + mw],
                                xwin(xt[k][s], 128, l0c, dh, dw),
                                start=first, stop=(t == 8 and k == 3),
                            )
                            first = False
                    evict_to_out(acc, mw, [(0, mw, mo)], b, g0)

            def emit_tail(b, s, l0c):
                if not rem9:
                    return
                q0 = l0c * WP + 1
                N = 7 * WP
                g0 = l0c - 1 + SLAB_ROWS * s
                acc = ps.tile([rem9, 7 * 56], F32, name="acc", tag="ps")
                accv = acc[:].rearrange("p (r w) -> p r w", w=56)
                first = True
                for t in CENTER_FIRST:
                    dh, dw = t // 3, t % 3
                    nc.tensor.matmul(
                        accv, selt[:NE, t, :], xwin(ze[s], NE, l0c, dh, dw),
                        start=first, stop=(t == 8),
                    )
                    first = False
                evict_to_out(acc, rem9, [(0, rem9, M0F)], b, g0)

            def emit_b1(b, s, l0c):
                q0 = l0c * WP + 1
                N = 7 * WP
                g0 = l0c - 1 + SLAB_ROWS * s
                for (mo, mw) in _tiles_of(M1):
                    acc = ps.tile([mw, 7 * 56], F32, name="acc", tag="ps")
                    accv = acc[:].rearrange("p (r w) -> p r w", w=56)
                    for k in range(4):
                        nc.tensor.matmul(
                            accv, wt1[:, k, mo : mo + mw], xwin(xt[k][s], 128, l0c, 1, 1),
                            start=(k == 0), stop=(k == 3),
                        )
                    evict_to_out(acc, mw, [(0, mw, off1 + mo)], b, g0)

            def emit_b2c2(b, s, l0c):
                q0 = l0c * WP + 1
                N = 7 * WP
                g0 = l0c - 1 + SLAB_ROWS * s
                for o, (mo, mw) in enumerate(_tiles_of(M2)):
                    acc = ps.tile([mw, 7 * 56], F32, name="acc", tag="ps")
                    accv = acc[:].rearrange("p (r w) -> p r w", w=56)
                    first = True
                    for t in CENTER_FIRST:
                        dh, dw = t // 3, t % 3
                        for k in range(2):
                            nc.tensor.matmul(
                                accv, wt33[:, k, t, mo : mo + mw],
                                xwin(z1[k][s], 128, l0c, dh, dw),
                                start=first, stop=(t == 8 and k == 1),
                            )
                            first = False
                    st = stage.tile([mw, 7 * 56], F32, name="st", tag="st")
                    nc.scalar.activation(
                        st[:], acc[:],
                        AF.Identity,
                        bias=part[:mw, cols["bias2"] + o : cols["bias2"] + o + 1],
                    )
                    nc.sync.dma_start(
                        out_ext.ap()[b, off2 + mo : off2 + mo + mw, g0 : g0 + 7, :],
                        st[:].rearrange("p (r w) -> p r w", w=56),
                    )

            # ---- pipelined emission schedule ----
            slabs = [(b, s) for b in range(BL) for s in range(2)]
            pieces, tiles = emit_x(*slabs[0])
            xq = {0: (pieces, tiles)}
            for idx, (b, s) in enumerate(slabs):
                x_pieces, xs_tiles = xq.pop(idx)
                if idx == 0:
                    load_big_weights()
                if idx == 1:
                    emit_pad_memsets(1)
                emit_copies(b, s, x_pieces, xs_tiles)
                emit_mids(b, s, xs_tiles)
                emit_pool(b, s)
                if idx + 1 < len(slabs):
                    xq[idx + 1] = emit_x(*slabs[idx + 1])
                if idx == 0:
                    # slab 0: run small-weight branches first so the large
                    # conv weights have time to stream in behind the x pieces
                    for l0c in OUT_STARTS:
                        emit_b1(b, s, l0c)
                    for l0c in OUT_STARTS:
                        emit_tail(b, s, l0c)
                    for l0c in OUT_STARTS:
                        emit_b2c2(b, s, l0c)
                    for l0c in OUT_STARTS:
                        emit_b0F(b, s, l0c)
                else:
                    for l0c in OUT_STARTS:
                        emit_b0F(b, s, l0c)
                        emit_tail(b, s, l0c)
                        emit_b1(b, s, l0c)
                    for l0c in OUT_STARTS:
                        emit_b2c2(b, s, l0c)

    nc.compile()
    return nc


# ---------------- entry point ----------------

def kernel(x, w_main, w_1x1, w31, bn31, w33, bn33, wa1, bna1, bna2, fuse_weight, c_score):
    global LAST_EXEC_NS
    arrs, counts, jperm = _prep(
        w_main, w_1x1, w31, bn31, w33, bn33, wa1, bna1, bna2, fuse_weight, c_score
    )
    if counts not in _CACHE:
        _CACHE[counts] = _build(counts)
    nc = _CACHE[counts]

    x = np.ascontiguousarray(x, dtype=np.float32)
    in_maps = []
    for i in range(NCORES):
        m = {"x": np.ascontiguousarray(x[BL * i : BL * (i + 1)])}
        m.update(arrs)
        in_maps.append(m)

    res = run_bass_kernel_spmd(nc, in_maps, list(range(NCORES)), trace=PROFILE)
    LAST_EXEC_NS = res.exec_time_ns

    full = np.empty((B, C, H, W), np.float32)
    for i in range(NCORES):
        full[BL * i : BL * (i + 1)] = res.results[i]["out"]
    out = np.empty_like(full)
    out[:, jperm] = full
    return out
